# revision 15
# baseline (speedup 1.0000x reference)
"""Multi-head attention (b=4, n=2048, dim=768, 12 heads) on 8 TRN2 NeuronCores.

Sharding: core c handles batch c//2 and head-group c%2 (6 of 12 heads).  Each
core computes its heads' contribution projected through its slice of Wo and
returns a partial [2048, 768] f32 output; the host sums core pairs and adds
the bias.  No on-device collectives needed.

Per-core kernel (all TensorE data bf16, accumulation f32):
  P1: KT/QT = W^T x^T feature-major per head-pair f (rows 0:64 = head 2f,
      64:128 = head 2f+1), V token-major with a ones column at 64 per head
      block (OP row 64 = softmax denominator for free).
  P2: per (head-pair f, i-block, j-chunk c): scores TRANSPOSED ST[j,i] = K Q^T
      (K=64) for BOTH heads into one [128, 1024] PSUM tile; the two score
      matmuls use PE row-halves 0:64 / 64:128 (tile_position), which the PE
      executes concurrently -> 2x throughput at K=64; score emission is
      batched in j-chunk pairs to reduce PE mode-turnaround.  Each ST tile's
      exp is split ACROSS BOTH engines concurrently: ACT runs Exp on columns
      [0:616] (bf16 out), DVE covers [616:1024] with a Schraudolph bit trick
      (one tensor_scalar: int16(round(s*log2(e)*128 + 16248.6)) whose bits
      ARE bf16(exp(s)); ~1.8% rms per weight, washes out in softmax).  This
      halves the exp latency that gates ST buffer reuse and balances engine
      load.  attnV lags 4 j-chunks behind the scores (hiding exp latency),
      accumulates OP[f,i] per head in PSUM over the 16 j-chunks, and drains
      2 per step once scores finish (taper) so the accumulator group closes
      early -- otherwise the OPS copy head-of-line-blocks ACT's queue into
      the next block.  PSUM: 3x2-bank score tiles + 2 accumulator banks = 8.
  P3: per 128 tokens: PE-transpose OP (bf16) to token-major, multiply by 1/l,
      PE-transpose back, project through Wo with PSUM accumulation; 3-stage
      software pipeline.
"""
import os
import sys
import types
import numpy as np
import ml_dtypes

B, N, DIM = 4, 2048, 768
HEADS, DH = 12, 64
HPC = 6                # heads per core
FPC = HPC * DH         # 384 features per core
NCORES = 8
KC = DIM // 128        # 6 contraction chunks
FT = 3                 # head-pairs per core
NT = N // 128          # 16 j-chunks of 128
IBS = 512              # i-block size
IB = N // IBS          # 4 i-blocks
BF16 = ml_dtypes.bfloat16

SC = 0.125
C1B = SC * 184.66496   # DVE bits: log2(e)*128 * logit scale
C2B = 16256.0 - 7.4    # bf16 exponent bias - Schraudolph sigma
SPL = 616              # exp split point: ACT does [0:SPL], DVE [SPL:1024]

_cache = {}
last_exec_time_ns = None


def _install_ntff_hook():
    try:
        import antenv.axon_hooks  # noqa: F401
        return
    except ImportError:
        pass
    from trn_agent_boot.trn_boot import _ntff_profile_via_ctypes
    hook = _ntff_profile_via_ctypes('/opt/axon/libaxon_pjrt.so')
    mod = types.ModuleType('antenv.axon_hooks')
    mod.get_axon_ntff_profile_hook = lambda: hook
    import antenv
    sys.modules['antenv.axon_hooks'] = mod
    antenv.axon_hooks = mod


def _build_nc():
    from contextlib import ExitStack
    from concourse import bacc
    import concourse.mybir as mybir
    from concourse.tile import TileContext
    from concourse.masks import make_identity
    from concourse.bass import broadcast_tensor_aps

    dt = mybir.dt
    EXP = mybir.ActivationFunctionType.Exp
    MUL, ADD = mybir.AluOpType.mult, mybir.AluOpType.add

    nc = bacc.Bacc("TRN2", target_bir_lowering=False, debug=False,
                   num_devices=NCORES)
    xT = nc.dram_tensor("xT", [DIM, N], dt.bfloat16, kind="ExternalInput").ap()
    wq = nc.dram_tensor("wq", [DIM, FPC], dt.bfloat16, kind="ExternalInput").ap()
    wk = nc.dram_tensor("wk", [DIM, FPC], dt.bfloat16, kind="ExternalInput").ap()
    wv = nc.dram_tensor("wv", [DIM, FPC], dt.bfloat16, kind="ExternalInput").ap()
    wo = nc.dram_tensor("wo", [FPC, DIM], dt.bfloat16, kind="ExternalInput").ap()
    out = nc.dram_tensor("out", [N, DIM], dt.float32, kind="ExternalOutput").ap()

    with TileContext(nc) as tc, ExitStack() as ctx:
        const = ctx.enter_context(tc.tile_pool(name="const", bufs=1))
        id_bf = const.tile([128, 128], dt.bfloat16, tag="idb")
        make_identity(nc, id_bf)

        inp = ctx.enter_context(tc.tile_pool(name="inp", bufs=1))
        xts2 = [[inp.tile([128, N // 2], dt.bfloat16, tag=f"xt{k}_{hf}",
                          name=f"xt{k}_{hf}") for hf in range(2)]
                for k in range(KC)]
        wqs = [inp.tile([128, FPC], dt.bfloat16, tag=f"wq{k}", name=f"wq{k}")
               for k in range(KC)]
        wks = [inp.tile([128, FPC], dt.bfloat16, tag=f"wk{k}", name=f"wk{k}")
               for k in range(KC)]
        wvs = [inp.tile([128, FPC], dt.bfloat16, tag=f"wv{k}", name=f"wv{k}")
               for k in range(KC)]
        wos = [inp.tile([128, DIM], dt.bfloat16, tag=f"wo{f}", name=f"wo{f}")
               for f in range(FT)]
        for k in range(KC):
            nc.sync.dma_start(out=xts2[k][0][:],
                              in_=xT[k * 128:(k + 1) * 128, 0:N // 2])
            nc.scalar.dma_start(out=wvs[k][:], in_=wv[k * 128:(k + 1) * 128, :])
        for k in range(KC):
            nc.sync.dma_start(out=xts2[k][1][:],
                              in_=xT[k * 128:(k + 1) * 128, N // 2:N])
        for k in range(KC):
            nc.sync.dma_start(out=wks[k][:], in_=wk[k * 128:(k + 1) * 128, :])
            nc.sync.dma_start(out=wqs[k][:], in_=wq[k * 128:(k + 1) * 128, :])
        for f in range(FT):
            nc.scalar.dma_start(out=wos[f][:], in_=wo[f * 128:(f + 1) * 128, :])

        kqv = ctx.enter_context(tc.tile_pool(name="kqv", bufs=1))
        KT = [kqv.tile([128, N], dt.bfloat16, tag=f"kt{f}", name=f"kt{f}")
              for f in range(FT)]
        QT = [kqv.tile([128, N], dt.bfloat16, tag=f"qt{f}", name=f"qt{f}")
              for f in range(FT)]
        VP = [kqv.tile([128, HPC * 128], dt.bfloat16, tag=f"vp{t}", name=f"vp{t}")
              for t in range(NT)]
        opsb = ctx.enter_context(tc.tile_pool(name="opsb", bufs=1))
        OPS = [[opsb.tile([65, IBS], dt.bfloat16, tag=f"op{h}_{ib}",
                          name=f"op{h}_{ib}") for ib in range(IB)]
               for h in range(HPC)]

        # ---- P1: projections ----
        for t in range(NT):
            nc.vector.memset(
                VP[t].rearrange("p (h c) -> p h c", c=128)[:, :, 64:65], 1.0)
        with tc.tile_pool(name="p1ps", bufs=6, space="PSUM") as p1:
            for t in range(NT):
                ps = p1.tile([128, FPC], dt.float32, tag="p1", name=f"vps{t}")
                for k in range(KC):
                    nc.tensor.matmul(
                        ps[:],
                        lhsT=xts2[k][t // 8][:, (t % 8) * 128:(t % 8 + 1) * 128],
                        rhs=wvs[k][:], start=(k == 0), stop=(k == KC - 1))
                nc.vector.tensor_copy(
                    VP[t].rearrange("p (h c) -> p h c", c=128)[:, :, 0:64],
                    ps.rearrange("p (h c) -> p h c", c=64))
            for W, DST in ((wks, KT), (wqs, QT)):
                for f in range(FT):
                    for q in range(N // 512):
                        ps = p1.tile([128, 512], dt.float32, tag="p1",
                                     name=f"kqps{f}_{q}")
                        for k in range(KC):
                            nc.tensor.matmul(
                                ps[:], lhsT=W[k][:, f * 128:(f + 1) * 128],
                                rhs=xts2[k][q // 2][:, (q % 2) * 512:
                                                    (q % 2 + 1) * 512],
                                start=(k == 0), stop=(k == KC - 1))
                        nc.scalar.copy(DST[f][:, q * 512:(q + 1) * 512], ps[:])

        # ---- P2: paired scores + alternating-engine exp + attnV ----
        LAG = 4
        with tc.tile_pool(name="p2st", bufs=3, space="PSUM") as p2st, \
                tc.tile_pool(name="p2op", bufs=2, space="PSUM") as p2op, \
                tc.tile_pool(name="exbp", bufs=LAG + 2) as exbp:
            for f in range(FT):
                for ib in range(IB):
                    opE = p2op.tile([128, IBS], dt.float32, tag="op",
                                    name=f"opE{f}_{ib}")
                    opO = p2op.tile([128, IBS], dt.float32, tag="op",
                                    name=f"opO{f}_{ib}")
                    pend = []
                    scq = []
                    for c in range(NT + 2):
                        if c < NT:
                            st = p2st.tile([128, 2 * IBS], dt.float32,
                                           tag="st", name=f"st{f}_{ib}_{c}")
                            scq.append((st, c))
                            if c % 2 == 1:
                                for stx, cc in scq:
                                    nc.tensor.matmul(
                                        stx[:, 0:IBS],
                                        lhsT=KT[f][0:64,
                                                   cc * 128:(cc + 1) * 128],
                                        rhs=QT[f][0:64,
                                                  ib * IBS:(ib + 1) * IBS],
                                        start=True, stop=True)
                                    nc.tensor.matmul(
                                        stx[:, IBS:2 * IBS],
                                        lhsT=KT[f][64:128,
                                                   cc * 128:(cc + 1) * 128],
                                        rhs=QT[f][64:128,
                                                  ib * IBS:(ib + 1) * IBS],
                                        start=True, stop=True)
                                for stx, cc in scq:
                                    ex = exbp.tile([128, 2 * IBS], dt.bfloat16,
                                                   tag="exb",
                                                   name=f"ex{f}_{ib}_{cc}")
                                    nc.scalar.activation(ex[:, 0:SPL],
                                                         stx[:, 0:SPL],
                                                         EXP, scale=SC)
                                    nc.vector.tensor_scalar(
                                        ex[:].bitcast(dt.int16)
                                        [:, SPL:2 * IBS],
                                        stx[:, SPL:2 * IBS], C1B, C2B,
                                        MUL, ADD)
                                    pend.append((ex, cc))
                                scq = []
                        ndr = (1 if c >= LAG else 0) + (1 if c >= NT else 0)
                        for _ in range(ndr):
                            ex, jc = pend.pop(0)
                            nc.tensor.matmul(
                                opE[:],
                                lhsT=VP[jc][:, (2 * f) * 128:
                                             (2 * f + 1) * 128],
                                rhs=ex[:, 0:IBS],
                                start=(jc == 0), stop=(jc == NT - 1))
                            nc.tensor.matmul(
                                opO[:],
                                lhsT=VP[jc][:, (2 * f + 1) * 128:
                                             (2 * f + 2) * 128],
                                rhs=ex[:, IBS:2 * IBS],
                                start=(jc == 0), stop=(jc == NT - 1))
                    nc.scalar.copy(OPS[2 * f][ib][:], opE[0:65, :])
                    nc.vector.tensor_copy(OPS[2 * f + 1][ib][:], opO[0:65, :])

        # ---- P3: normalize + output projection (3-stage pipeline) ----
        with tc.tile_pool(name="p3tr", bufs=2, space="PSUM") as p3tr, \
                tc.tile_pool(name="p3tb", bufs=2, space="PSUM") as p3tb, \
                tc.tile_pool(name="p3pp", bufs=4, space="PSUM") as p3pp, \
                tc.tile_pool(name="otokp", bufs=4) as otokp, \
                tc.tile_pool(name="otnp", bufs=6) as otnp, \
                tc.tile_pool(name="linvp", bufs=3) as linvp, \
                tc.tile_pool(name="outst", bufs=4) as outst:
            otoks = {}
            otns = {}

            def tr_block(isub):
                ib, col = isub // 4, (isub % 4) * 128
                trp = p3tr.tile([128, HPC * 66], dt.bfloat16, tag="tr",
                                name=f"trp{isub}")
                for h in range(HPC):
                    nc.tensor.transpose(trp[:, h * 66:h * 66 + 65],
                                        OPS[h][ib][0:65, col:col + 128],
                                        id_bf[0:65, 0:65])
                trv = trp.rearrange("p (h c) -> p h c", c=66)
                linv6 = linvp.tile([128, HPC, 1], dt.float32, tag="l6",
                                   name=f"l6{isub}")
                nc.vector.reciprocal(linv6[:], trv[:, :, 64:65])
                otok = otokp.tile([128, FPC], dt.bfloat16, tag="otok",
                                  name=f"otok{isub}")
                a, b = broadcast_tensor_aps(trv[:, :, 0:64], linv6[:])
                nc.vector.tensor_mul(
                    otok.rearrange("p (h c) -> p h c", c=64), a, b)
                otoks[isub] = otok

            def tb_block(isub):
                otok = otoks.pop(isub)
                tbp = p3tb.tile([128, FPC], dt.bfloat16, tag="tb",
                                name=f"tbp{isub}")
                for f in range(FT):
                    nc.tensor.transpose(tbp[:, f * 128:(f + 1) * 128],
                                        otok[:, f * 128:(f + 1) * 128],
                                        id_bf[:])
                otn = otnp.tile([128, FPC], dt.bfloat16, tag="otn",
                                name=f"otn{isub}")
                nc.vector.tensor_copy(otn[:], tbp[:])
                otns[isub] = otn

            def proj_block(isub):
                otn = otns.pop(isub)
                ob = outst.tile([128, DIM], dt.float32, tag="ob",
                                name=f"ob{isub}")
                for half in range(2):
                    pp = p3pp.tile([128, DIM // 2], dt.float32, tag="pp",
                                   name=f"pp{isub}_{half}")
                    for f in range(FT):
                        nc.tensor.matmul(
                            pp[:], lhsT=otn[:, f * 128:(f + 1) * 128],
                            rhs=wos[f][:, half * 384:(half + 1) * 384],
                            start=(f == 0), stop=(f == FT - 1))
                    nc.scalar.copy(ob[:, half * 384:(half + 1) * 384], pp[:])
                nc.sync.dma_start(out=out[isub * 128:(isub + 1) * 128, :],
                                  in_=ob[:])

            lags = [(tr_block, 0), (tb_block, 1), (proj_block, 2)]
            for step in range(NT + 2):
                for fn, lag in lags:
                    i = step - lag
                    if 0 <= i < NT:
                        fn(i)

    nc.finalize()
    return nc


def _get_nc():
    if "nc" not in _cache:
        _cache["nc"] = _build_nc()
    return _cache["nc"]


def kernel(x, Wq, Wk, Wv, Wo, bo):
    global last_exec_time_ns
    x = np.asarray(x, dtype=np.float32)
    Wq = np.asarray(Wq, dtype=np.float32)
    Wk = np.asarray(Wk, dtype=np.float32)
    Wv = np.asarray(Wv, dtype=np.float32)
    Wo = np.asarray(Wo, dtype=np.float32)
    bo = np.asarray(bo, dtype=np.float32)

    trace = bool(os.environ.get("BASS_KERNEL_TRACE"))
    if trace:
        _install_ntff_hook()
        import concourse.bass_utils as bass_utils
        bass_utils.upload_artifacts = lambda tmpdir: tmpdir

    nc = _get_nc()
    in_maps = []
    for c in range(NCORES):
        bi, hg = divmod(c, 2)
        s = slice(hg * FPC, (hg + 1) * FPC)
        in_maps.append({
            "xT": np.ascontiguousarray(x[bi].T).astype(BF16),
            "wq": np.ascontiguousarray(Wq[:, s]).astype(BF16),
            "wk": np.ascontiguousarray(Wk[:, s]).astype(BF16),
            "wv": np.ascontiguousarray(Wv[:, s]).astype(BF16),
            "wo": np.ascontiguousarray(Wo[s, :]).astype(BF16),
        })

    from concourse.bass_utils import run_bass_kernel_spmd
    res = run_bass_kernel_spmd(nc, in_maps, list(range(NCORES)), trace=trace)
    last_exec_time_ns = res.exec_time_ns

    parts = [res.results[c]["out"] for c in range(NCORES)]
    full = np.empty((B, N, DIM), np.float32)
    for bi in range(B):
        full[bi] = parts[2 * bi] + parts[2 * bi + 1] + bo[None, :]
    return full



# revision 16
# speedup vs baseline: 1.0038x; 1.0038x over previous
"""Multi-head attention (b=4, n=2048, dim=768, 12 heads) on 8 TRN2 NeuronCores.

Sharding: core c handles batch c//2 and head-group c%2 (6 of 12 heads).  Each
core computes its heads' contribution projected through its slice of Wo and
returns a partial [2048, 768] f32 output; the host sums core pairs and adds
the bias.  No on-device collectives needed.

Per-core kernel (all TensorE data bf16, accumulation f32):
  P1: KT/QT = W^T x^T feature-major per head-pair f (rows 0:64 = head 2f,
      64:128 = head 2f+1), V token-major with a ones column at 64 per head
      block (OP row 64 = softmax denominator for free).
  P2: per (head-pair f, i-block, j-chunk c): scores TRANSPOSED ST[j,i] = K Q^T
      (K=64) for BOTH heads into one [128, 1024] PSUM tile; the two score
      matmuls use PE row-halves 0:64 / 64:128 (tile_position), which the PE
      executes concurrently -> 2x throughput at K=64; score emission is
      batched in j-chunk pairs to reduce PE mode-turnaround.  Each ST tile's
      exp is split ACROSS BOTH engines concurrently: ACT runs Exp on columns
      [0:616] (bf16 out), DVE covers [616:1024] with a Schraudolph bit trick
      (one tensor_scalar: int16(round(s*log2(e)*128 + 16248.6)) whose bits
      ARE bf16(exp(s)); ~1.8% rms per weight, washes out in softmax).  This
      halves the exp latency that gates ST buffer reuse and balances engine
      load.  attnV lags 4 j-chunks behind the scores (hiding exp latency),
      accumulates OP[f,i] per head in PSUM over the 16 j-chunks, and drains
      2 per step once scores finish (taper) so the accumulator group closes
      early -- otherwise the OPS copy head-of-line-blocks ACT's queue into
      the next block.  PSUM: 3x2-bank score tiles + 2 accumulator banks = 8.
  P3: per 128 tokens: PE-transpose OP (bf16) to token-major, multiply by 1/l,
      PE-transpose back, project through Wo with PSUM accumulation; 3-stage
      software pipeline.
"""
import os
import sys
import types
import numpy as np
import ml_dtypes

B, N, DIM = 4, 2048, 768
HEADS, DH = 12, 64
HPC = 6                # heads per core
FPC = HPC * DH         # 384 features per core
NCORES = 8
KC = DIM // 128        # 6 contraction chunks
FT = 3                 # head-pairs per core
NT = N // 128          # 16 j-chunks of 128
IBS = 512              # i-block size
IB = N // IBS          # 4 i-blocks
BF16 = ml_dtypes.bfloat16

SC = 0.125
C1B = SC * 184.66496   # DVE bits: log2(e)*128 * logit scale
C2B = 16256.0 - 7.4    # bf16 exponent bias - Schraudolph sigma
SPL = 616              # exp split point: ACT does [0:SPL], DVE [SPL:1024]

_cache = {}
last_exec_time_ns = None


def _install_ntff_hook():
    try:
        import antenv.axon_hooks  # noqa: F401
        return
    except ImportError:
        pass
    from trn_agent_boot.trn_boot import _ntff_profile_via_ctypes
    hook = _ntff_profile_via_ctypes('/opt/axon/libaxon_pjrt.so')
    mod = types.ModuleType('antenv.axon_hooks')
    mod.get_axon_ntff_profile_hook = lambda: hook
    import antenv
    sys.modules['antenv.axon_hooks'] = mod
    antenv.axon_hooks = mod


def _build_nc():
    from contextlib import ExitStack
    from concourse import bacc
    import concourse.mybir as mybir
    from concourse.tile import TileContext
    from concourse.masks import make_identity
    from concourse.bass import broadcast_tensor_aps

    dt = mybir.dt
    EXP = mybir.ActivationFunctionType.Exp
    MUL, ADD = mybir.AluOpType.mult, mybir.AluOpType.add

    nc = bacc.Bacc("TRN2", target_bir_lowering=False, debug=False,
                   num_devices=NCORES)
    xT = nc.dram_tensor("xT", [DIM, N], dt.bfloat16, kind="ExternalInput").ap()
    wq = nc.dram_tensor("wq", [DIM, FPC], dt.bfloat16, kind="ExternalInput").ap()
    wk = nc.dram_tensor("wk", [DIM, FPC], dt.bfloat16, kind="ExternalInput").ap()
    wv = nc.dram_tensor("wv", [DIM, FPC], dt.bfloat16, kind="ExternalInput").ap()
    wo = nc.dram_tensor("wo", [FPC, DIM], dt.bfloat16, kind="ExternalInput").ap()
    out = nc.dram_tensor("out", [N, DIM], dt.float32, kind="ExternalOutput").ap()

    with TileContext(nc) as tc, ExitStack() as ctx:
        const = ctx.enter_context(tc.tile_pool(name="const", bufs=1))
        id_bf = const.tile([128, 128], dt.bfloat16, tag="idb")
        make_identity(nc, id_bf)

        inp = ctx.enter_context(tc.tile_pool(name="inp", bufs=1))
        xts2 = [[inp.tile([128, N // 2], dt.bfloat16, tag=f"xt{k}_{hf}",
                          name=f"xt{k}_{hf}") for hf in range(2)]
                for k in range(KC)]
        wqs = [inp.tile([128, FPC], dt.bfloat16, tag=f"wq{k}", name=f"wq{k}")
               for k in range(KC)]
        wks = [inp.tile([128, FPC], dt.bfloat16, tag=f"wk{k}", name=f"wk{k}")
               for k in range(KC)]
        wvs = [inp.tile([128, FPC], dt.bfloat16, tag=f"wv{k}", name=f"wv{k}")
               for k in range(KC)]
        wos = [inp.tile([128, DIM], dt.bfloat16, tag=f"wo{f}", name=f"wo{f}")
               for f in range(FT)]
        for k in range(KC):
            nc.sync.dma_start(out=xts2[k][0][:],
                              in_=xT[k * 128:(k + 1) * 128, 0:N // 2])
            nc.scalar.dma_start(out=wvs[k][:], in_=wv[k * 128:(k + 1) * 128, :])
        for k in range(KC):
            nc.sync.dma_start(out=xts2[k][1][:],
                              in_=xT[k * 128:(k + 1) * 128, N // 2:N])
        for k in range(KC):
            nc.sync.dma_start(out=wks[k][:], in_=wk[k * 128:(k + 1) * 128, :])
            nc.sync.dma_start(out=wqs[k][:], in_=wq[k * 128:(k + 1) * 128, :])
        for f in range(FT):
            nc.scalar.dma_start(out=wos[f][:], in_=wo[f * 128:(f + 1) * 128, :])

        kqv = ctx.enter_context(tc.tile_pool(name="kqv", bufs=1))
        KT = [kqv.tile([128, N], dt.bfloat16, tag=f"kt{f}", name=f"kt{f}")
              for f in range(FT)]
        QT = [kqv.tile([128, N], dt.bfloat16, tag=f"qt{f}", name=f"qt{f}")
              for f in range(FT)]
        VP = [kqv.tile([128, HPC * 128], dt.bfloat16, tag=f"vp{t}", name=f"vp{t}")
              for t in range(NT)]
        opsb = ctx.enter_context(tc.tile_pool(name="opsb", bufs=1))
        OPS = [[opsb.tile([65, IBS], dt.bfloat16, tag=f"op{h}_{ib}",
                          name=f"op{h}_{ib}") for ib in range(IB)]
               for h in range(HPC)]

        # ---- P1: projections ----
        for t in range(NT):
            nc.vector.memset(
                VP[t].rearrange("p (h c) -> p h c", c=128)[:, :, 64:65], 1.0)
        with tc.tile_pool(name="p1ps", bufs=8, space="PSUM") as p1:
            # k-outer V projection: the first matmul only needs x-chunk 0 +
            # wv chunk 0 (~1us into the DMA ramp) instead of all 6 chunks
            # (~4.3us), so the PE starts (and HAM-warms) ~3us earlier.
            for tg in range(2):
                pss = [p1.tile([128, FPC], dt.float32, tag="p1",
                               name=f"vps{tg}_{i}") for i in range(8)]
                for k in range(KC):
                    for i in range(8):
                        nc.tensor.matmul(
                            pss[i][:],
                            lhsT=xts2[k][tg][:, i * 128:(i + 1) * 128],
                            rhs=wvs[k][:], start=(k == 0), stop=(k == KC - 1))
                for i in range(8):
                    t = tg * 8 + i
                    nc.vector.tensor_copy(
                        VP[t].rearrange("p (h c) -> p h c", c=128)[:, :, 0:64],
                        pss[i].rearrange("p (h c) -> p h c", c=64))
            for W, DST in ((wks, KT), (wqs, QT)):
                for f in range(FT):
                    for q in range(N // 512):
                        ps = p1.tile([128, 512], dt.float32, tag="p1",
                                     name=f"kqps{f}_{q}")
                        for k in range(KC):
                            nc.tensor.matmul(
                                ps[:], lhsT=W[k][:, f * 128:(f + 1) * 128],
                                rhs=xts2[k][q // 2][:, (q % 2) * 512:
                                                    (q % 2 + 1) * 512],
                                start=(k == 0), stop=(k == KC - 1))
                        nc.scalar.copy(DST[f][:, q * 512:(q + 1) * 512], ps[:])

        # ---- P2: paired scores + alternating-engine exp + attnV ----
        LAG = 4
        with tc.tile_pool(name="p2st", bufs=3, space="PSUM") as p2st, \
                tc.tile_pool(name="p2op", bufs=2, space="PSUM") as p2op, \
                tc.tile_pool(name="exbp", bufs=LAG + 2) as exbp:
            for f in range(FT):
                for ib in range(IB):
                    opE = p2op.tile([128, IBS], dt.float32, tag="op",
                                    name=f"opE{f}_{ib}")
                    opO = p2op.tile([128, IBS], dt.float32, tag="op",
                                    name=f"opO{f}_{ib}")
                    pend = []
                    scq = []
                    for c in range(NT + 2):
                        if c < NT:
                            st = p2st.tile([128, 2 * IBS], dt.float32,
                                           tag="st", name=f"st{f}_{ib}_{c}")
                            scq.append((st, c))
                            if c % 2 == 1:
                                for stx, cc in scq:
                                    nc.tensor.matmul(
                                        stx[:, 0:IBS],
                                        lhsT=KT[f][0:64,
                                                   cc * 128:(cc + 1) * 128],
                                        rhs=QT[f][0:64,
                                                  ib * IBS:(ib + 1) * IBS],
                                        start=True, stop=True)
                                    nc.tensor.matmul(
                                        stx[:, IBS:2 * IBS],
                                        lhsT=KT[f][64:128,
                                                   cc * 128:(cc + 1) * 128],
                                        rhs=QT[f][64:128,
                                                  ib * IBS:(ib + 1) * IBS],
                                        start=True, stop=True)
                                for stx, cc in scq:
                                    ex = exbp.tile([128, 2 * IBS], dt.bfloat16,
                                                   tag="exb",
                                                   name=f"ex{f}_{ib}_{cc}")
                                    nc.scalar.activation(ex[:, 0:SPL],
                                                         stx[:, 0:SPL],
                                                         EXP, scale=SC)
                                    nc.vector.tensor_scalar(
                                        ex[:].bitcast(dt.int16)
                                        [:, SPL:2 * IBS],
                                        stx[:, SPL:2 * IBS], C1B, C2B,
                                        MUL, ADD)
                                    pend.append((ex, cc))
                                scq = []
                        ndr = (1 if c >= LAG else 0) + (1 if c >= NT else 0)
                        for _ in range(ndr):
                            ex, jc = pend.pop(0)
                            nc.tensor.matmul(
                                opE[:],
                                lhsT=VP[jc][:, (2 * f) * 128:
                                             (2 * f + 1) * 128],
                                rhs=ex[:, 0:IBS],
                                start=(jc == 0), stop=(jc == NT - 1))
                            nc.tensor.matmul(
                                opO[:],
                                lhsT=VP[jc][:, (2 * f + 1) * 128:
                                             (2 * f + 2) * 128],
                                rhs=ex[:, IBS:2 * IBS],
                                start=(jc == 0), stop=(jc == NT - 1))
                    nc.scalar.copy(OPS[2 * f][ib][:], opE[0:65, :])
                    nc.vector.tensor_copy(OPS[2 * f + 1][ib][:], opO[0:65, :])

        # ---- P3: normalize + output projection (3-stage pipeline) ----
        with tc.tile_pool(name="p3tr", bufs=2, space="PSUM") as p3tr, \
                tc.tile_pool(name="p3tb", bufs=2, space="PSUM") as p3tb, \
                tc.tile_pool(name="p3pp", bufs=4, space="PSUM") as p3pp, \
                tc.tile_pool(name="otokp", bufs=4) as otokp, \
                tc.tile_pool(name="otnp", bufs=6) as otnp, \
                tc.tile_pool(name="linvp", bufs=3) as linvp, \
                tc.tile_pool(name="outst", bufs=4) as outst:
            otoks = {}
            otns = {}

            def tr_block(isub):
                ib, col = isub // 4, (isub % 4) * 128
                trp = p3tr.tile([128, HPC * 66], dt.bfloat16, tag="tr",
                                name=f"trp{isub}")
                for h in range(HPC):
                    nc.tensor.transpose(trp[:, h * 66:h * 66 + 65],
                                        OPS[h][ib][0:65, col:col + 128],
                                        id_bf[0:65, 0:65])
                trv = trp.rearrange("p (h c) -> p h c", c=66)
                linv6 = linvp.tile([128, HPC, 1], dt.float32, tag="l6",
                                   name=f"l6{isub}")
                nc.vector.reciprocal(linv6[:], trv[:, :, 64:65])
                otok = otokp.tile([128, FPC], dt.bfloat16, tag="otok",
                                  name=f"otok{isub}")
                a, b = broadcast_tensor_aps(trv[:, :, 0:64], linv6[:])
                nc.vector.tensor_mul(
                    otok.rearrange("p (h c) -> p h c", c=64), a, b)
                otoks[isub] = otok

            def tb_block(isub):
                otok = otoks.pop(isub)
                tbp = p3tb.tile([128, FPC], dt.bfloat16, tag="tb",
                                name=f"tbp{isub}")
                for f in range(FT):
                    nc.tensor.transpose(tbp[:, f * 128:(f + 1) * 128],
                                        otok[:, f * 128:(f + 1) * 128],
                                        id_bf[:])
                otn = otnp.tile([128, FPC], dt.bfloat16, tag="otn",
                                name=f"otn{isub}")
                nc.vector.tensor_copy(otn[:], tbp[:])
                otns[isub] = otn

            def proj_block(isub):
                otn = otns.pop(isub)
                ob = outst.tile([128, DIM], dt.float32, tag="ob",
                                name=f"ob{isub}")
                for half in range(2):
                    pp = p3pp.tile([128, DIM // 2], dt.float32, tag="pp",
                                   name=f"pp{isub}_{half}")
                    for f in range(FT):
                        nc.tensor.matmul(
                            pp[:], lhsT=otn[:, f * 128:(f + 1) * 128],
                            rhs=wos[f][:, half * 384:(half + 1) * 384],
                            start=(f == 0), stop=(f == FT - 1))
                    nc.scalar.copy(ob[:, half * 384:(half + 1) * 384], pp[:])
                nc.sync.dma_start(out=out[isub * 128:(isub + 1) * 128, :],
                                  in_=ob[:])

            lags = [(tr_block, 0), (tb_block, 1), (proj_block, 2)]
            for step in range(NT + 2):
                for fn, lag in lags:
                    i = step - lag
                    if 0 <= i < NT:
                        fn(i)

    nc.finalize()
    return nc


def _get_nc():
    if "nc" not in _cache:
        _cache["nc"] = _build_nc()
    return _cache["nc"]


def kernel(x, Wq, Wk, Wv, Wo, bo):
    global last_exec_time_ns
    x = np.asarray(x, dtype=np.float32)
    Wq = np.asarray(Wq, dtype=np.float32)
    Wk = np.asarray(Wk, dtype=np.float32)
    Wv = np.asarray(Wv, dtype=np.float32)
    Wo = np.asarray(Wo, dtype=np.float32)
    bo = np.asarray(bo, dtype=np.float32)

    trace = bool(os.environ.get("BASS_KERNEL_TRACE"))
    if trace:
        _install_ntff_hook()
        import concourse.bass_utils as bass_utils
        bass_utils.upload_artifacts = lambda tmpdir: tmpdir

    nc = _get_nc()
    in_maps = []
    for c in range(NCORES):
        bi, hg = divmod(c, 2)
        s = slice(hg * FPC, (hg + 1) * FPC)
        in_maps.append({
            "xT": np.ascontiguousarray(x[bi].T).astype(BF16),
            "wq": np.ascontiguousarray(Wq[:, s]).astype(BF16),
            "wk": np.ascontiguousarray(Wk[:, s]).astype(BF16),
            "wv": np.ascontiguousarray(Wv[:, s]).astype(BF16),
            "wo": np.ascontiguousarray(Wo[s, :]).astype(BF16),
        })

    from concourse.bass_utils import run_bass_kernel_spmd
    res = run_bass_kernel_spmd(nc, in_maps, list(range(NCORES)), trace=trace)
    last_exec_time_ns = res.exec_time_ns

    parts = [res.results[c]["out"] for c in range(NCORES)]
    full = np.empty((B, N, DIM), np.float32)
    for bi in range(B):
        full[bi] = parts[2 * bi] + parts[2 * bi + 1] + bo[None, :]
    return full

